# revision 1
# baseline (speedup 1.0000x reference)
"""Trainium2 Bass kernel for nn_Attention_5145370821223.

Computation (per batch b of 16, heads H=6, tokens N=512, dim 78, dh 13):
    qkv = x @ W_qkv ; dots = q k^T / sqrt(13), masked by m_i & m_j
    attn = softmax(dots) * 1.0 + 0.5 * adj * (m_i & m_j)
    y = (attn @ v) @ W_out + b_out

Strategy: data-parallel over batch (2 batches per NeuronCore x 8 cores).
On-core pipeline works in the "transposed" orientation so that the softmax
matrix P^T = exp(dots^T) comes out of the scalar engine already laid out with
the contraction (j) axis on partitions for the attention*V matmuls.

Mask handling (validated bit-for-bit against the jax reference semantics):
  dots'^T[j,i] = m_i*(q_i.k_j/sqrt(dh)) + m_i*(30*m_j - 30)
  - multiplicative m_i on the q-side makes fully-masked rows exactly constant,
    so softmax gives the reference's uniform 1/512 rows;
  - the additive -30*(1-m_j) term (folded into the k-side matmul as an extra
    contraction row) kills masked keys: exp(-30)~9e-14.
  rowsums come free via ones-columns appended to V; the softmax division is
  deferred to the [128, 84] attention output (diag scaling commutes).
  adj term: 0.5*m_i*(adj @ (v*m_j)), shared across heads; adj is transposed
  on the host so its j axis lands on partitions.
"""

import os
import numpy as np
import ml_dtypes

H, DH, DIM = 6, 13, 78
LA, LG = 1.0, 0.5
B, N = 16, 512
SCALE = DH ** -0.5
NEG = 30.0
NCORES = 8
BPC = B // NCORES          # batches per core
NT = N // 128              # 128-token tiles per sequence
HDA = 14                   # dh + 1 (ones column) per head in V_aug
PASS_A = [0, 1, 2, 3]
PASS_B = [4, 5]

_CACHE = {}


# ---------------------------------------------------------------------------
# Workaround: this container's walrus rejects the multi-wait Drain that
# TileContext emits at exit ("Too many sync wait commands"). Split the waits
# into individual wait_ge instructions on the SP engine before a bare drain.
def _apply_tile_patch(tile_mod, ScopedClock):
    def _patched(self, tick_clock, wait_clock):
        nc = self.nc
        drain_inst = nc.sync.drain()
        wait_clock.add_sem_waits(
            drain_inst.ins, ScopedClock({None: tick_clock.global_clock})
        )
        mi = drain_inst.ins
        waits = list(mi.sync_info.on_wait)
        if len(waits) > 1:
            handles = {s.name: s for s in self.sems.allocated().values()}
            engines = [nc.sync, nc.vector, nc.scalar, nc.tensor, nc.gpsimd]
            kept = []
            k = 0
            for w in waits:
                h = handles.get(w.ant_name)
                if h is None:
                    kept.append(w)
                    continue
                engines[k % len(engines)].wait_ge(h, w.wait_value)
                k += 1
            mi.sync_info.on_wait = kept
        nc.all_engine_barrier()
        assert self.sems is not None
        popped = nc._tile_sem_poison_stack.pop()
        assert popped is self._sem_poison
        nc.clear_and_free_semaphores(list(self.sems.allocated().values()))
        nc.all_engine_barrier()

    tile_mod.TileContext._drain_and_barrier = _patched


def _split_waits(nc, mybir):
    """This walrus build only encodes one sem-wait per instruction; hoist
    extra waits onto same-engine NoOps inserted right before the owner."""
    k = 0
    for f in nc.m.functions:
        for bb in f.blocks:
            out = []
            changed = False
            for inst in bb.instructions:
                si = inst.sync_info
                waits = list(si.on_wait) if si is not None else []
                if len(waits) > 1:
                    changed = True
                    for w in waits[:-1]:
                        n = mybir.InstNoOp(name=f"I-wsplit-{k}", ins=[], outs=[])
                        k += 1
                        n.engine = inst.engine
                        n.sync_info = mybir.SyncInfo(on_wait=[w], on_update=[])
                        out.append(n)
                    si.on_wait = [waits[-1]]
                out.append(inst)
            if changed:
                bb.instructions = out


# ---------------------------------------------------------------------------
def _host_weights(W_qkv, W_out, b_out):
    """Rearrange weights into the stationary layouts the kernel uses."""
    W = W_qkv.reshape(DIM, H, 3, DH).astype(np.float32)

    def qk_stack(heads, kind):
        w = np.zeros((80, 128), np.float32)
        for g, h in enumerate(heads):
            c0 = 32 * g
            if kind == "q":
                w[0:DIM, c0:c0 + DH] = W[:, h, 0, :] * SCALE
                w[79, c0 + DH] = 1.0          # ones input row -> m_i after mask mul
            else:
                w[0:DIM, c0:c0 + DH] = W[:, h, 1, :]
                w[78, c0 + DH] = NEG          # mask row  -> +30*m_j
                w[79, c0 + DH] = -NEG         # ones row  -> -30
        return w

    wqa = qk_stack(PASS_A, "q")
    wqb = qk_stack(PASS_B, "q")
    wka = qk_stack(PASS_A, "k")
    wkb = qk_stack(PASS_B, "k")

    wv = np.zeros((80, H * HDA), np.float32)
    for h in range(H):
        wv[0:DIM, h * HDA:h * HDA + DH] = W[:, h, 2, :]

    wo = np.zeros((80, DIM), np.float32)
    wo[0:DIM, :] = W_out.astype(np.float32)
    wo[78, :] = b_out.astype(np.float32)
    # single packed stationary-weights tensor: [80, 4*128 + 84 + 78]
    return np.concatenate([wqa, wqb, wka, wkb, wv, wo], axis=1)


def _build_bass(walrus_patches=True):
    import concourse.bass as bass
    import concourse.mybir as mybir
    import concourse.tile as tile
    from concourse.vector_clock import ScopedClock
    from concourse.masks import make_identity

    if walrus_patches:
        _apply_tile_patch(tile, ScopedClock)

    f32 = mybir.dt.float32
    f32r = mybir.dt.float32r
    bf16 = mybir.dt.bfloat16
    AF = mybir.ActivationFunctionType
    OP = mybir.AluOpType

    nc = bass.Bass()
    WCOLS = 4 * 128 + H * HDA + DIM
    xaug_d = nc.dram_tensor("xaug", [BPC, 80, N], f32r, kind="ExternalInput")
    maskf = nc.dram_tensor("maskf", [BPC, N], f32r, kind="ExternalInput")
    maskc = nc.dram_tensor("maskc", [BPC, N], f32, kind="ExternalInput")
    adjt = nc.dram_tensor("adjt", [BPC, N, N], f32, kind="ExternalInput")
    wall_d = nc.dram_tensor("wall", [80, WCOLS], f32r, kind="ExternalInput")
    yout = nc.dram_tensor("yout", [BPC, N, DIM], f32, kind="ExternalOutput")

    with tile.TileContext(nc) as tc:
        with (
            tc.tile_pool(name="consts", bufs=1) as consts,
            tc.tile_pool(name="bpool", bufs=2) as bpool,
            tc.tile_pool(name="ptpA", bufs=2 * NT) as ptpA,
            tc.tile_pool(name="ptpB", bufs=2 * NT) as ptpB,
            tc.tile_pool(name="spool", bufs=3) as spool,
            tc.tile_pool(name="opool", bufs=8) as opool,
            tc.tile_pool(name="ps_small", bufs=2, space="PSUM") as ps_small,
            tc.tile_pool(name="ps_dotsA", bufs=1, space="PSUM") as ps_dotsA,
            tc.tile_pool(name="ps_dotsB", bufs=1, space="PSUM") as ps_dotsB,
        ):
            # --- constants (weight DMAs are emitted inside phase1(0) after
            # the latency-critical x/mask loads so the DMA queue serves those
            # first) ---
            identity = consts.tile([128, 128], bf16)
            make_identity(nc, identity)
            # tiny warm-up exp: hoists the one-time ~2.7us ACT table load
            # for the exp set into the initial DMA wait instead of the
            # first real exp's critical path
            warm = consts.tile([128, 1], f32, tag="warm")
            nc.vector.memset(warm, 0.0)
            nc.scalar.activation(warm[:], warm[:], AF.Exp)
            ones_col = consts.tile([1, 128], f32r)
            nc.gpsimd.memset(ones_col[:].bitcast(f32), 1.0)
            wall = consts.tile([80, WCOLS], f32r, tag="wall")
            wqa = wall[:, 0:128]
            wqb = wall[:, 128:256]
            wka = wall[:, 256:384]
            wkb = wall[:, 384:512]
            wv = wall[:, 512:512 + H * HDA]
            wo = wall[0:79, 512 + H * HDA:WCOLS]

            def phase1(b):
                # ---- latency-critical loads (mask first: the m_bc chain
                # only needs maskrow + ones_col) ----
                # xTaug rows: 0..77 x^T, 78 mask, 79 ones (host-built)
                maskrow = bpool.tile([1, N], f32r, tag="maskrow")
                nc.sync.dma_start(maskrow[:], maskf[b:b + 1, :])
                xTaug = bpool.tile([80, N], f32r, tag="xTaug")
                nc.sync.dma_start(xTaug[:], xaug_d[b])
                if b == 0:
                    nc.sync.dma_start(wall[:], wall_d[:])

                # ---- mask broadcast [128, N] via PE outer product ----
                psb = ps_small.tile([128, N], f32, tag="ps")
                nc.tensor.matmul(psb[:], ones_col[:], maskrow[:])
                m_bc = bpool.tile([128, N], bf16, tag="m_bc")
                (nc.scalar.copy if b == 0 else nc.vector.tensor_copy)(
                    m_bc[:], psb[:])

                # ---- projections ----
                # batch 0's plain copies go on the scalar engine (idle before
                # the first exp; exp table already loaded by the warm-up);
                # batch 1's stay on DVE (ACT is busy with exps then).
                cp = nc.vector.tensor_copy

                def project(w_sb, out_name, masked, copier=None):
                    ps = ps_small.tile([128, N], f32, tag="ps")
                    nc.tensor.matmul(ps[:], w_sb[:], xTaug[:])
                    dst = bpool.tile([128, N], f32r, tag=out_name)
                    if masked:
                        nc.vector.tensor_tensor(
                            dst[:], ps[:], m_bc[:], op=OP.mult)
                    else:
                        (copier or cp)(dst[:], ps[:])
                    return dst

                qsA = project(wqa, "qsA", True)
                # ksA's input is ready before the first exp's, so its copy can
                # ride the otherwise-idle scalar engine without blocking exps
                ksA = project(wka, "ksA", False,
                              copier=nc.scalar.copy if b == 0 else None)

                # ---- dots^T + exp; pass A (heads 0-3) / pass B (heads 4,5)
                # use separate psum pools so chunk jt+1's matmuls overlap the
                # exp of chunk jt and the scalar engine never starves ----
                ptsA, ptsB = [], []
                qsB = ksB = None
                for jt in range(NT):
                    jsl = slice(jt * 128, (jt + 1) * 128)
                    psdA = ps_dotsA.tile([128, 4, N], f32, tag="psdA")
                    for g in range(4):
                        c = 32 * g
                        nc.tensor.matmul(
                            psdA[:, g, :], ksA[c:c + DH + 1, jsl],
                            qsA[c:c + DH + 1, :], tile_position=(c, 0))
                    ptA = ptpA.tile([128, 4, N], bf16, tag="ptA")
                    nc.scalar.activation(
                        ptA[:].rearrange("p h n -> p (h n)"),
                        psdA[:].rearrange("p h n -> p (h n)"), AF.Exp)
                    ptsA.append(ptA)

                    if jt == 0:
                        qsB = project(wqb, "qsB", True)
                        ksB = project(wkb, "ksB", False)

                    psdB = ps_dotsB.tile([128, 2, N], f32, tag="psdB")
                    for g in range(2):
                        c = 32 * g
                        nc.tensor.matmul(
                            psdB[:, g, :], ksB[c:c + DH + 1, jsl],
                            qsB[c:c + DH + 1, :], tile_position=(c, 0))
                    ptB = ptpB.tile([128, 2, N], bf16, tag="ptB")
                    nc.scalar.activation(
                        ptB[:].rearrange("p h n -> p (h n)"),
                        psdB[:].rearrange("p h n -> p (h n)"), AF.Exp)
                    ptsB.append(ptB)

                mcol = bpool.tile([128, NT], f32, tag="mcol")
                nc.sync.dma_start(
                    mcol[:], maskc[b].rearrange("(t p) -> p t", p=128))

                # ---- V projection + V natural + masked V (phase-2 inputs) --
                psv = ps_small.tile([H * HDA, N], f32, tag="ps")
                nc.tensor.matmul(psv[:], wv[:], xTaug[:])
                vT = bpool.tile([H * HDA, N], bf16, tag="vT")
                cp(vT[:], psv[:])
                va = bpool.tile([128, NT, H * HDA], bf16, tag="va")
                vm = bpool.tile([128, NT, H * DH], f32, tag="vm")
                for t in range(NT):
                    psvt = ps_small.tile([128, H * HDA], bf16, tag="ps")
                    nc.tensor.transpose(
                        psvt[:], vT[:, t * 128:(t + 1) * 128],
                        identity[0:H * HDA, 0:H * HDA])
                    nc.vector.tensor_copy(va[:, t, :], psvt[:])
                    # masked V for the adjacency term, in f32 from the psum
                    # (keeps the dominant adj@V path at full precision)
                    nc.vector.tensor_scalar(
                        vm[:, t, :].rearrange("p (h c) -> p h c", c=DH),
                        psvt[:].rearrange("p (h c) -> p h c", c=HDA)[:, :, 0:DH],
                        mcol[:, t:t + 1], LG,
                        op0=OP.mult, op1=OP.mult)
                # ones columns (rowsum trick), after the copies (WAW ordered)
                nc.gpsimd.memset(
                    va[:].rearrange("p t (h c) -> p t h c", c=HDA)[:, :, :, DH:HDA],
                    1.0)

                # adjacency load is only needed by phase 2; emitted last (and
                # on the SWDGE queue) so it overlaps the exp phase.
                adjs = bpool.tile([128, NT, N], f32, tag="adjs")
                nc.sync.dma_start(
                    adjs[:], adjt[b].rearrange("(t p) i -> p t i", p=128))
                return dict(ptsA=ptsA, ptsB=ptsB, va=va, vm=vm, adjs=adjs,
                            mcol=mcol)

            def phase2(b, st):
                ptsA, ptsB, va, vm, adjs, mcol = (
                    st["ptsA"], st["ptsB"], st["va"], st["vm"], st["adjs"],
                    st["mcol"])
                outT = bpool.tile([79, N], f32r, tag="outT")
                ysb = bpool.tile([128, NT, DIM], f32, tag="ysb")
                # attn buffer: col 78 is a ones column so the transpose
                # carries the bias row for the output projection.
                attn = bpool.tile([128, NT, 79], bf16, tag="attn")
                nc.gpsimd.memset(attn[:, :, 78:79], 1.0)
                # AV + adj accumulation in psum, split into jt halves so the
                # psum slot is only held for ~2 exp periods; the halves are
                # summed in sbuf during the combine.
                osbs = []
                for it in range(4):
                    isl = slice(it * 128, (it + 1) * 128)
                    for half in range(1):
                        jts = (0, 1, 2, 3) if half == 0 else ()
                        pso = ps_small.tile([128, 162], f32, tag="ps")
                        # one accumulation group for the whole pso bank:
                        # start only on the first matmul, stop on the last
                        # (interleaved per-region groups corrupt each other --
                        # the start flag's zero region is bank-granular).
                        last = len(jts) - 1
                        for jj, jt in enumerate(jts):
                            for h in range(4):
                                nc.tensor.matmul(
                                    pso[:, h * HDA:(h + 1) * HDA],
                                    ptsA[jt][:, h, isl],
                                    va[:, jt, h * HDA:(h + 1) * HDA],
                                    start=(jj == 0 and h == 0), stop=False,
                                    skip_group_check=True)
                            for h in range(4, H):
                                nc.tensor.matmul(
                                    pso[:, h * HDA:(h + 1) * HDA],
                                    ptsB[jt][:, h - 4, isl],
                                    va[:, jt, h * HDA:(h + 1) * HDA],
                                    start=False, stop=False,
                                    skip_group_check=True)
                            nc.tensor.matmul(
                                pso[:, 84:84 + H * DH],
                                adjs[:, jt, isl],
                                vm[:, jt, :],
                                start=False, stop=(jj == last),
                                skip_group_check=True)
                        osb = opool.tile([128, 162], f32, tag="osb")
                        (nc.scalar.copy if b == BPC - 1
                         else nc.vector.tensor_copy)(osb[:], pso[:])
                        osbs.append(osb)

                for it in range(4):
                    isl = slice(it * 128, (it + 1) * 128)
                    osb = osbs[it]
                    pv_heads = osb[:, 0:84].rearrange("p (h c) -> p h c", c=HDA)
                    rs6 = spool.tile([128, H], f32, tag="rs6")
                    nc.vector.reciprocal(
                        rs6[:].unsqueeze(2), pv_heads[:, :, DH:HDA])
                    attn_pv = spool.tile([128, H * DH], f32, tag="attn_pv")
                    nc.gpsimd.tensor_tensor(
                        attn_pv[:].rearrange("p (h c) -> p h c", c=DH),
                        pv_heads[:, :, 0:DH],
                        rs6[:].unsqueeze(2).broadcast_to([128, H, DH]),
                        op=OP.mult)
                    nc.vector.scalar_tensor_tensor(
                        attn[:, it, 0:H * DH], osb[:, 84:84 + H * DH],
                        mcol[:, it:it + 1],
                        attn_pv[:], op0=OP.mult, op1=OP.add)
                    # transpose + output projection (bias via ones column)
                    tailcp = nc.vector.tensor_copy
                    psa = ps_small.tile([79, 128], bf16, tag="ps")
                    nc.tensor.transpose(psa[:], attn[:, it, :], identity[:])
                    tailcp(outT[:, isl], psa[:])
                    psy = ps_small.tile([128, DIM], f32, tag="ps")
                    nc.tensor.matmul(psy[:], outT[:, isl], wo[:])
                    tailcp(ysb[:, it, :], psy[:])
                    nc.sync.dma_start(
                        yout[b].rearrange("(t p) f -> p t f", p=128)[:, it, :],
                        ysb[:, it, :])

            # batch-level software pipeline: both batches' setup/dots/exp are
            # emitted before either batch's AV/output phase, so the scalar
            # engine (the bottleneck: fused exps) runs back-to-back while
            # PE/DVE fill in AV and output work underneath.
            states = [phase1(b) for b in range(BPC)]
            for b in range(BPC):
                phase2(b, states[b])

    if walrus_patches:
        _split_waits(nc, mybir)
    return nc


def _prep_inputs(x, mask, adjacency_mat, W_qkv, W_out, b_out):
    x = np.asarray(x, np.float32)
    maskf = np.ascontiguousarray(np.asarray(mask, np.float32))
    adj = np.asarray(adjacency_mat, np.float32)
    adjt = np.ascontiguousarray(adj.transpose(0, 2, 1))
    wall = _host_weights(
        np.asarray(W_qkv, np.float32), np.asarray(W_out, np.float32),
        np.asarray(b_out, np.float32))
    # xaug: rows 0..77 x^T, row 78 mask, row 79 ones (built on host)
    xaug = np.zeros((B, 80, N), np.float32)
    xaug[:, 0:DIM, :] = x.transpose(0, 2, 1)
    xaug[:, 78, :] = maskf.astype(np.float32)
    xaug[:, 79, :] = 1.0
    in_maps = []
    for c in range(NCORES):
        s = slice(c * BPC, (c + 1) * BPC)
        in_maps.append({
            "xaug": np.ascontiguousarray(xaug[s]),
            "maskf": np.ascontiguousarray(maskf[s].astype(np.float32)),
            "maskc": np.ascontiguousarray(maskf[s].astype(np.float32)),
            "adjt": np.ascontiguousarray(adjt[s]),
            "wall": wall,
        })
    return in_maps


LAST_EXEC_NS = None
LAST_RESULT = None


def kernel(x, mask, adjacency_mat, W_qkv, W_out, b_out):
    global LAST_EXEC_NS, LAST_RESULT
    from concourse.bass_utils import run_bass_kernel_spmd

    if "nc" not in _CACHE:
        _CACHE["nc"] = _build_bass()
    nc = _CACHE["nc"]

    in_maps = _prep_inputs(x, mask, adjacency_mat, W_qkv, W_out, b_out)
    trace = bool(int(os.environ.get("KERNEL_TRACE", "0")))
    res = run_bass_kernel_spmd(
        nc, in_maps, core_ids=list(range(NCORES)), trace=trace)
    LAST_EXEC_NS = res.exec_time_ns
    LAST_RESULT = res
    y = np.concatenate([res.results[c]["yout"] for c in range(NCORES)], axis=0)
    return np.ascontiguousarray(y.astype(np.float32))

